# revision 1
# baseline (speedup 1.0000x reference)
"""Trainium2 Bass kernel for nn_ComplexCrossAttention.

Strategy:
- Data-parallel over batch B=8 across 8 NeuronCores (one batch element each,
  no collectives).
- Complex linears are folded into single real matmuls on stacked
  real/imag feature-major activations Z = [re; im] with host-prestacked
  weights [[Wr^T, Wi^T], [-Wi^T, Wr^T]]  (out = W_stack^T-contract over 2*Din).
- Attention per head: scores are computed TRANSPOSED (St[k,q]) so softmax-exp
  runs straight out of PSUM with no transposes; the key-axis softmax sum is a
  ones-vector matmul on the PE; normalization by 1/denom is deferred to the
  AV-output eviction (per-column broadcast multiply).
- exp() needs no max-subtraction for this problem's input distribution
  (|scores| < ~10 << 88).
- All matmuls run as float32r (full-rate fp32 on the PE; moving dim kept
  >= 256).
"""

import sys

for _p in ("/opt/trn_rl_repo",):
    if _p not in sys.path:
        sys.path.insert(0, _p)

import numpy as np

import concourse.bass as bass
import concourse.mybir as mybir
import concourse.tile as tile
from concourse import bacc
from concourse.bass_utils import run_bass_kernel_spmd

FP32R = mybir.dt.float32r
FP32 = mybir.dt.float32
AF = mybir.ActivationFunctionType
OP = mybir.AluOpType

B, S, D = 8, 512, 1024
NH, DH = 16, 64
HID = 4096
T = S
N_CORES = 8
D2 = 2 * D       # 2048 stacked features
H2 = 2 * HID     # 8192 stacked hidden
KC_D = D2 // 128   # 16 contraction chunks of the model dim
MC_D = D2 // 128   # 16 output chunks of the model dim
MC_H = H2 // 128   # 64 chunks of the hidden dim
EPS = 1e-5

# fc/proj hidden chunk order: [re half1, im half1, re half2, im half2] so each
# token-independent MLP "hidden half" is a contiguous chunk range pairing
# re chunk j with im chunk j+16.
MC_ORDER = (
    list(range(0, 16)) + list(range(32, 48))
    + list(range(16, 32)) + list(range(48, 64))
)


def _build_nc():
    nc = bacc.Bacc(None, target_bir_lowering=False, debug=False)

    zq_d = nc.dram_tensor("zq", [KC_D, 128, T], FP32R, kind="ExternalInput")
    zx_d = nc.dram_tensor("zx", [KC_D, 128, T], FP32R, kind="ExternalInput")
    wq_d = nc.dram_tensor("wq", [MC_D, 128, KC_D, 128], FP32R, kind="ExternalInput")
    wk_d = nc.dram_tensor("wk", [MC_D, 128, KC_D, 128], FP32R, kind="ExternalInput")
    wv_d = nc.dram_tensor("wv", [KC_D, 128, D2], FP32R, kind="ExternalInput")
    wfc_d = nc.dram_tensor("wfc", [MC_H, 128, KC_D, 128], FP32R, kind="ExternalInput")
    wpj_d = nc.dram_tensor("wpj", [MC_D, 128, MC_H, 128], FP32R, kind="ExternalInput")
    bq_d = nc.dram_tensor("bq", [MC_D, 128], FP32, kind="ExternalInput")
    bk_d = nc.dram_tensor("bk", [MC_D, 128], FP32, kind="ExternalInput")
    bv_d = nc.dram_tensor("bv", [1, D2], FP32, kind="ExternalInput")
    bfc_d = nc.dram_tensor("bfc", [MC_H, 128], FP32, kind="ExternalInput")
    bp_d = nc.dram_tensor("bp", [MC_D, 128], FP32, kind="ExternalInput")
    lng_d = nc.dram_tensor("lng", [128, 48], FP32, kind="ExternalInput")
    lnb_d = nc.dram_tensor("lnb", [128, 48], FP32, kind="ExternalInput")
    y_d = nc.dram_tensor("y", [MC_D, 128, T], FP32R, kind="ExternalOutput")

    with tile.TileContext(nc) as tc:
        consts_cm = tc.tile_pool(name="consts", bufs=1)
        consts = consts_cm.__enter__()

        ones_f = consts.tile([128, 1], FP32)
        nc.vector.memset(ones_f[:], 1.0)
        eps_t = consts.tile([128, 1], FP32)
        nc.vector.memset(eps_t[:], EPS)
        ones_r = consts.tile([128, 1], FP32R)
        nc.vector.tensor_copy(ones_r[:], ones_f[:])
        bq_s = consts.tile([128, MC_D], FP32)
        nc.sync.dma_start(bq_s[:], bq_d.rearrange("m p -> p m"))
        bk_s = consts.tile([128, MC_D], FP32)
        nc.sync.dma_start(bk_s[:], bk_d.rearrange("m p -> p m"))
        bfc_s = consts.tile([128, MC_H], FP32)
        nc.sync.dma_start(bfc_s[:], bfc_d.rearrange("m p -> p m"))
        bp_s = consts.tile([128, MC_D], FP32)
        nc.sync.dma_start(bp_s[:], bp_d.rearrange("m p -> p m"))
        bv_row = consts.tile([1, D2], FP32)
        nc.sync.dma_start(bv_row[:], bv_d[:])
        bv_b = consts.tile([128, D2], FP32)
        nc.gpsimd.partition_broadcast(bv_b[:], bv_row[:])
        lng_s = consts.tile([128, 48], FP32)
        nc.sync.dma_start(lng_s[:], lng_d[:])
        lnb_s = consts.tile([128, 48], FP32)
        nc.sync.dma_start(lnb_s[:], lnb_d[:])

        def ln_gb(idx, comp, c8):
            j = idx * 16 + comp * 8 + c8
            return lng_s[:, j:j + 1], lnb_s[:, j:j + 1]

        # ---- long-lived activation pools (manually scoped) ----
        zx_cm = tc.tile_pool(name="zx", bufs=1)
        zx_pool = zx_cm.__enter__()
        zx_s = zx_pool.tile([128, KC_D, T], FP32R, name="zx_s")
        nc.sync.dma_start(zx_s[:], zx_d.rearrange("c p t -> p c t"))

        o_cm = tc.tile_pool(name="op", bufs=1)
        o_pool = o_cm.__enter__()
        o_s = o_pool.tile([128, MC_D, T], FP32R, name="o_s")

        q_cm = tc.tile_pool(name="qp", bufs=1)
        q_pool = q_cm.__enter__()
        q_s = q_pool.tile([128, NH, T], FP32R, name="q_s")

        # =============== Phase A: Q projection (feature-major) ===============
        with (
            tc.tile_pool(name="zqa", bufs=1) as zqa_pool,
            tc.tile_pool(name="wqp", bufs=3) as wq_pool,
            tc.tile_pool(name="psA", bufs=4, space="PSUM") as psA,
        ):
            zq_a = zqa_pool.tile([128, KC_D, T], FP32R, name="zq_a")
            nc.sync.dma_start(zq_a[:], zq_d.rearrange("c p t -> p c t"))
            for mc in range(MC_D):
                wt = wq_pool.tile([128, KC_D, 128], FP32R, tag="wq")
                nc.sync.dma_start(wt[:], wq_d[mc])
                ps = psA.tile([128, T], FP32, tag="psA")
                for kc in range(KC_D):
                    nc.tensor.matmul(
                        ps[:], wt[:, kc, :], zq_a[:, kc, :],
                        start=(kc == 0), stop=(kc == KC_D - 1),
                    )
                nc.scalar.activation(
                    q_s[:, mc, :], ps[:], AF.Identity, bias=bq_s[:, mc:mc + 1]
                )

        # =============== Phase B: attention, head-streamed ===============
        with (
            tc.tile_pool(name="wkp", bufs=2) as wk_pool,
            tc.tile_pool(name="wvp", bufs=1) as wv_pool,
            tc.tile_pool(name="kp", bufs=4) as k_pool,
            tc.tile_pool(name="vp", bufs=2) as v_pool,
            tc.tile_pool(name="ep", bufs=10) as e_pool,
            tc.tile_pool(name="stp", bufs=2) as st_pool,
            tc.tile_pool(name="ttp", bufs=1) as tt_pool,
            tc.tile_pool(name="recp", bufs=2) as rec_pool,
            tc.tile_pool(name="bcp", bufs=2) as bc_pool,
            tc.tile_pool(name="psK", bufs=1, space="PSUM") as psK,
            tc.tile_pool(name="psV", bufs=1, space="PSUM") as psV,
            tc.tile_pool(name="psS", bufs=2, space="PSUM") as psS,
            tc.tile_pool(name="psO", bufs=2, space="PSUM") as psO,
            tc.tile_pool(name="psD", bufs=2, space="PSUM") as psD,
        ):
            v_cur = None
            for h in range(NH):
                hp, par = divmod(h, 2)
                if par == 0:
                    # V1 projection for the head pair (token-major) + V2 assembly
                    wvt = wv_pool.tile([128, KC_D, 256], FP32R, tag="wv")
                    nc.sync.dma_start(
                        wvt[:],
                        wv_d[:, :, hp * 256:(hp + 1) * 256].rearrange("c p f -> p c f"),
                    )
                    v_cur = v_pool.tile([128, 4, 512], FP32R, tag="v")
                    for tcb in range(4):
                        psv = psV.tile([128, 256], FP32, tag="psV")
                        for kc in range(KC_D):
                            nc.tensor.matmul(
                                psv[:],
                                zx_s[:, kc, tcb * 128:(tcb + 1) * 128],
                                wvt[:, kc, :],
                                start=(kc == 0), stop=(kc == KC_D - 1),
                            )
                        for sub in range(2):
                            hh = hp * 2 + sub
                            base = sub * 256
                            nc.vector.tensor_tensor(
                                v_cur[:, tcb, base:base + 128],
                                psv[:, sub * 128:(sub + 1) * 128],
                                bv_b[:, hh * 128:(hh + 1) * 128],
                                OP.add,
                            )
                            # V2 = [-Vi | Vr]
                            nc.vector.tensor_scalar_mul(
                                v_cur[:, tcb, base + 128:base + 192],
                                v_cur[:, tcb, base + 64:base + 128],
                                -1.0,
                            )
                            nc.vector.tensor_copy(
                                v_cur[:, tcb, base + 192:base + 256],
                                v_cur[:, tcb, base:base + 64],
                            )

                # K1 = [Kr; -Ki] projection (feature-major); K2 = [Ki; Kr]
                # is a partition swap + negate of K1 (saves 16 matmuls/head)
                wkt = wk_pool.tile([128, KC_D, 128], FP32R, tag="wk")
                nc.sync.dma_start(wkt[:], wk_d[h])
                k1 = k_pool.tile([128, T], FP32R, tag="k")
                ps = psK.tile([128, T], FP32, tag="psK")
                for kc in range(KC_D):
                    nc.tensor.matmul(
                        ps[:], wkt[:, kc, :], zx_s[:, kc, :],
                        start=(kc == 0), stop=(kc == KC_D - 1),
                    )
                nc.scalar.activation(
                    k1[:], ps[:], AF.Identity, bias=bk_s[:, h:h + 1]
                )
                k2 = k_pool.tile([128, T], FP32R, tag="k")
                nc.sync.dma_start(k2[0:64, :], k1[64:128, :])
                nc.vector.tensor_scalar_mul(k2[0:64, :], k2[0:64, :], -1.0)
                nc.sync.dma_start(k2[64:128, :], k1[0:64, :])
                k_t = [k1, k2]

                # transposed scores + exp (comp 0: re via K1, comp 1: im via K2)
                e_tiles = [[None] * 4 for _ in range(2)]
                for comp in range(2):
                    for kc4 in range(4):
                        pss = psS.tile([128, T], FP32, tag="psS")
                        nc.tensor.matmul(
                            pss[:],
                            k_t[comp][:, kc4 * 128:(kc4 + 1) * 128],
                            q_s[:, h, :],
                            start=True, stop=True,
                        )
                        et = e_pool.tile([128, T], FP32R, tag="e")
                        nc.scalar.activation(et[:], pss[:], AF.Exp)
                        e_tiles[comp][kc4] = et

                # softmax denominators: ones-matmul over the key axis
                bc = []
                for comp in range(2):
                    psd = psD.tile([1, T], FP32, tag="psD")
                    for kc4 in range(4):
                        nc.tensor.matmul(
                            psd[:], ones_r[:], e_tiles[comp][kc4],
                            start=(kc4 == 0), stop=(kc4 == 3),
                        )
                    rec = rec_pool.tile([1, T], FP32, tag="rec")
                    nc.vector.reciprocal(rec[:], psd[:])
                    bct = bc_pool.tile([128, T], FP32, tag="bc")
                    nc.gpsimd.partition_broadcast(bct[:], rec[:])
                    bc.append(bct)

                # AV: two accumulation groups (er-part needs /dr, ei-part /di)
                pso = []
                for comp in range(2):
                    p = psO.tile([128, T], FP32, tag="psO")
                    for kc4 in range(4):
                        base = par * 256 + comp * 128
                        nc.tensor.matmul(
                            p[:],
                            v_cur[:, kc4, base:base + 128],
                            e_tiles[comp][kc4],
                            start=(kc4 == 0), stop=(kc4 == 3),
                        )
                    pso.append(p)

                # normalized eviction into natural-order O:
                # out = pso_r * (1/dr) + pso_i * (1/di), rows [Or(0:64); Oi(64:128)]
                c = h // 2
                stg = st_pool.tile([128, T], FP32R, tag="stg")
                ta = tt_pool.tile([128, T], FP32, tag="ta")
                tb = tt_pool.tile([128, T], FP32, tag="tb")
                if par == 0:
                    dsl, ssl = slice(0, 64), slice(64, 128)   # direct Or, shifted Oi
                else:
                    dsl, ssl = slice(64, 128), slice(0, 64)   # direct Oi, shifted Or
                nc.vector.tensor_tensor(ta[dsl, :], pso[0][dsl, :], bc[0][dsl, :], OP.mult)
                nc.vector.tensor_tensor(tb[dsl, :], pso[1][dsl, :], bc[1][dsl, :], OP.mult)
                nc.vector.tensor_tensor(ta[ssl, :], pso[0][ssl, :], bc[0][ssl, :], OP.mult)
                nc.vector.tensor_tensor(tb[ssl, :], pso[1][ssl, :], bc[1][ssl, :], OP.mult)
                nc.vector.tensor_tensor(stg[ssl, :], ta[ssl, :], tb[ssl, :], OP.add)
                if par == 0:
                    nc.vector.tensor_tensor(
                        o_s[0:64, c, :], ta[0:64, :], tb[0:64, :], OP.add
                    )
                    nc.sync.dma_start(o_s[0:64, 8 + c, :], stg[64:128, :])
                else:
                    nc.vector.tensor_tensor(
                        o_s[64:128, 8 + c, :], ta[64:128, :], tb[64:128, :], OP.add
                    )
                    nc.sync.dma_start(o_s[64:128, c, :], stg[0:64, :])

        q_cm.__exit__(None, None, None)

        # =============== Phase C: residuals + two layernorms ===============
        def layer_norm(src_fn, dst_fn, idx, psum_pool, small, bcast, sqp, width):
            """LN over the 1024 features of each of re (chunks 0-7) and
            im (chunks 8-15); src/dst_fn(c) -> [128, width] APs."""
            ps_sum = []
            for comp in range(2):
                p = psum_pool.tile([1, width], FP32, tag="lnps")
                for c8 in range(8):
                    nc.tensor.matmul(
                        p[:], ones_r[:], src_fn(comp * 8 + c8),
                        start=(c8 == 0), stop=(c8 == 7),
                    )
                ps_sum.append(p)
            stats = []
            for comp in range(2):
                mean = small.tile([1, width], FP32, tag="mean")
                nc.vector.tensor_scalar_mul(mean[:], ps_sum[comp][:], 1.0 / D)
                stats.append(mean)
            ps_sq = []
            for comp in range(2):
                p = psum_pool.tile([1, width], FP32, tag="lnps")
                for c8 in range(8):
                    sq = sqp.tile([128, width], FP32R, tag="sq")
                    srcc = src_fn(comp * 8 + c8)
                    nc.vector.tensor_tensor(sq[:], srcc, srcc, OP.mult)
                    nc.tensor.matmul(
                        p[:], ones_r[:], sq[:],
                        start=(c8 == 0), stop=(c8 == 7),
                    )
                ps_sq.append(p)
            bcs = []
            for comp in range(2):
                mean = stats[comp]
                msq = small.tile([1, width], FP32, tag="msq")
                nc.vector.tensor_scalar_mul(msq[:], ps_sq[comp][:], 1.0 / D)
                m2 = small.tile([1, width], FP32, tag="m2")
                nc.vector.tensor_tensor(m2[:], mean[:], mean[:], OP.mult)
                var = small.tile([1, width], FP32, tag="var")
                nc.vector.tensor_tensor(var[:], msq[:], m2[:], OP.subtract)
                sstd = small.tile([1, width], FP32, tag="sstd")
                nc.scalar.activation(sstd[:], var[:], AF.Sqrt, bias=eps_t[0:1, :])
                rstd = small.tile([1, width], FP32, tag="rstd")
                nc.vector.reciprocal(rstd[:], sstd[:])
                bm = bcast.tile([128, width], FP32, tag="bm")
                nc.gpsimd.partition_broadcast(bm[:], mean[:])
                br = bcast.tile([128, width], FP32, tag="br")
                nc.gpsimd.partition_broadcast(br[:], rstd[:])
                bcs.append((bm, br))
            for c in range(MC_D):
                comp = c // 8
                bm, br = bcs[comp]
                g_ap, b_ap = ln_gb(idx, comp, c % 8)
                tmp = sqp.tile([128, width], FP32, tag="lnt")
                nc.vector.tensor_tensor(tmp[:], src_fn(c), bm[:], OP.subtract)
                nc.vector.tensor_tensor(tmp[:], tmp[:], br[:], OP.mult)
                nc.vector.tensor_scalar(
                    dst_fn(c), tmp[:], g_ap, b_ap, OP.mult, OP.add
                )

        with (
            tc.tile_pool(name="zqc", bufs=1) as zqc_pool,
            tc.tile_pool(name="on1", bufs=1) as on1_pool,
            tc.tile_pool(name="lnsq", bufs=3) as sq_pool,
            tc.tile_pool(name="lnsm", bufs=1) as small_pool,
            tc.tile_pool(name="lnbc", bufs=2) as bc2_pool,
            tc.tile_pool(name="psC", bufs=2, space="PSUM") as psC,
        ):
            zq_c = zqc_pool.tile([128, KC_D, T], FP32R, name="zq_c")
            nc.sync.dma_start(zq_c[:], zq_d.rearrange("c p t -> p c t"))
            for c in range(MC_D):
                nc.vector.tensor_tensor(
                    o_s[:, c, :], o_s[:, c, :], zq_c[:, c, :], OP.add
                )
            on1_t = on1_pool.tile([128, MC_D, T], FP32R, name="on1")
            layer_norm(
                lambda c: o_s[:, c, :], lambda c: on1_t[:, c, :],
                0, psC, small_pool, bc2_pool, sq_pool, T,
            )
            for c in range(MC_D):
                nc.vector.tensor_tensor(
                    zx_s[:, c, :], zx_s[:, c, :], on1_t[:, c, :], OP.add
                )
            layer_norm(
                lambda c: zx_s[:, c, :], lambda c: zx_s[:, c, :],
                1, psC, small_pool, bc2_pool, sq_pool, T,
            )
        x2_s = zx_s   # LN2 ran in place; zx_s now holds x2
        part_s = o_s  # o_s contents are dead; reuse as c_proj accumulator

        # =============== Phase D: complex MLP (hidden-split) ===============
        with (
            tc.tile_pool(name="wfcp", bufs=2) as wfc_pool,
            tc.tile_pool(name="wpjp", bufs=2) as wpj_pool,
            tc.tile_pool(name="hp", bufs=1) as h_pool,
            tc.tile_pool(name="mrt", bufs=1) as mr_pool,
            tc.tile_pool(name="lnsq2", bufs=2) as sq2_pool,
            tc.tile_pool(name="lnsm2", bufs=1) as small2_pool,
            tc.tile_pool(name="lnbc2", bufs=1) as bc3_pool,
            tc.tile_pool(name="psF", bufs=4, space="PSUM") as psF,
            tc.tile_pool(name="psP", bufs=2, space="PSUM") as psP,
            tc.tile_pool(name="psC2", bufs=2, space="PSUM") as psC2,
        ):
            for th in range(2):
                h_t = h_pool.tile([128, 32, T], FP32R, tag="h")
                # c_fc for this hidden half
                for mcl in range(32):
                    mc = th * 32 + mcl
                    wt = wfc_pool.tile([128, KC_D, 128], FP32R, tag="wfc")
                    nc.sync.dma_start(wt[:], wfc_d[mc])
                    ps = psF.tile([128, T], FP32, tag="psF")
                    for kc in range(KC_D):
                        nc.tensor.matmul(
                            ps[:], wt[:, kc, :], x2_s[:, kc, :],
                            start=(kc == 0), stop=(kc == KC_D - 1),
                        )
                    nc.scalar.activation(
                        h_t[:, mcl, :], ps[:], AF.Identity, bias=bfc_s[:, mc:mc + 1]
                    )
                # modReLU (0.5 factor folded into wpj): hr <- hr + |h|
                for j in range(16):
                    hr = h_t[:, j, :]
                    hi = h_t[:, 16 + j, :]
                    t1 = mr_pool.tile([128, T], FP32, tag="mr1")
                    nc.vector.tensor_tensor(t1[:], hr, hr, OP.mult)
                    t2 = mr_pool.tile([128, T], FP32, tag="mr2")
                    nc.scalar.activation(t2[:], hi, AF.Square)
                    nc.vector.tensor_tensor(t1[:], t1[:], t2[:], OP.add)
                    nc.scalar.activation(t2[:], t1[:], AF.Sqrt)
                    nc.vector.tensor_tensor(hr, hr, t2[:], OP.add)
                # c_proj partial for this half
                for mc in range(MC_D):
                    ps = psP.tile([128, T], FP32, tag="psP")
                    for kq in range(2):
                        wt = wpj_pool.tile([128, 16, 128], FP32R, tag="wpj")
                        nc.sync.dma_start(
                            wt[:], wpj_d[mc][:, th * 32 + kq * 16:th * 32 + (kq + 1) * 16, :]
                        )
                        for kc in range(16):
                            nc.tensor.matmul(
                                ps[:], wt[:, kc, :], h_t[:, kq * 16 + kc, :],
                                start=(kq == 0 and kc == 0),
                                stop=(kq == 1 and kc == 15),
                            )
                    if th == 0:
                        nc.scalar.activation(part_s[:, mc, :], ps[:], AF.Copy)
                    else:
                        nc.vector.scalar_tensor_tensor(
                            part_s[:, mc, :], ps[:], bp_s[:, mc:mc + 1],
                            part_s[:, mc, :], OP.add, OP.add,
                        )
                        nc.vector.tensor_tensor(
                            part_s[:, mc, :], part_s[:, mc, :], x2_s[:, mc, :],
                            OP.add,
                        )

            # final layernorm (in place on part_s), then store
            layer_norm(
                lambda c: part_s[:, c, :], lambda c: part_s[:, c, :],
                2, psC2, small2_pool, bc3_pool, sq2_pool, T,
            )
            nc.sync.dma_start(y_d.rearrange("c p t -> p c t"), part_s[:])

        o_cm.__exit__(None, None, None)
        zx_cm.__exit__(None, None, None)
        consts_cm.__exit__(None, None, None)

    nc.compile()
    if not nc.is_finalized():
        nc.finalize()
    return nc


def _stackT(w):
    """[F, Din, 2] torch-layout complex weight -> [2*Din, 2*F] stacked lhsT."""
    wr = w[..., 0].astype(np.float32)
    wi = w[..., 1].astype(np.float32)
    top = np.concatenate([wr.T, wi.T], axis=1)
    bot = np.concatenate([-wi.T, wr.T], axis=1)
    return np.concatenate([top, bot], axis=0)


def _prep_weights(wq, bq, wk, bk, wv, bv, w_fc, b_fc, w_proj, b_proj, ln_g, ln_b):
    qcols = np.concatenate(
        [np.concatenate([np.arange(h * 64, h * 64 + 64),
                         1024 + np.arange(h * 64, h * 64 + 64)]) for h in range(NH)]
    )
    scale = np.float32(1.0 / np.sqrt(DH))

    sq = _stackT(wq) * scale
    wq_t = np.ascontiguousarray(
        sq[:, qcols].reshape(KC_D, 128, MC_D, 128).transpose(2, 1, 0, 3)
    )
    bq_l = (np.concatenate([bq[:, 0], bq[:, 1]]) * scale)[qcols]
    bq_a = np.ascontiguousarray(bq_l.reshape(MC_D, 128).astype(np.float32))

    sk = _stackT(wk)
    bkst = np.concatenate([bk[:, 0], bk[:, 1]]).astype(np.float32)
    wk_full = sk[:, qcols].copy()           # [2048, 2048]: per head [Kr | Ki]
    bk_l = bkst[qcols].copy()
    for h in range(NH):
        wk_full[:, h * 128 + 64:h * 128 + 128] *= -1.0   # -> [Kr | -Ki]
        bk_l[h * 128 + 64:h * 128 + 128] *= -1.0
    wk_t = np.ascontiguousarray(
        wk_full.reshape(KC_D, 128, MC_D, 128).transpose(2, 1, 0, 3)
    )
    bk_a = np.ascontiguousarray(bk_l.reshape(MC_D, 128))

    sv = _stackT(wv)
    wv_t = np.ascontiguousarray(sv[:, qcols].reshape(KC_D, 128, D2))
    bv_l = np.concatenate([bv[:, 0], bv[:, 1]]).astype(np.float32)[qcols]
    bv_a = np.ascontiguousarray(bv_l.reshape(1, D2))

    sfc = _stackT(w_fc)
    wfc_t = np.ascontiguousarray(
        sfc.reshape(KC_D, 128, MC_H, 128).transpose(2, 1, 0, 3)[MC_ORDER]
    )
    bfc_l = np.concatenate([b_fc[:, 0], b_fc[:, 1]]).astype(np.float32)
    bfc_a = np.ascontiguousarray(bfc_l.reshape(MC_H, 128)[MC_ORDER])

    spj = _stackT(w_proj) * np.float32(0.5)
    wpj_t = np.ascontiguousarray(
        spj.reshape(MC_H, 128, MC_D, 128)[MC_ORDER].transpose(2, 1, 0, 3)
    )
    bp_l = np.concatenate([b_proj[:, 0], b_proj[:, 1]]).astype(np.float32)
    bp_a = np.ascontiguousarray(bp_l.reshape(MC_D, 128))

    lng_a = np.ascontiguousarray(
        ln_g.astype(np.float32).reshape(3, 2, 8, 128).transpose(3, 0, 1, 2).reshape(128, 48)
    )
    lnb_a = np.ascontiguousarray(
        ln_b.astype(np.float32).reshape(3, 2, 8, 128).transpose(3, 0, 1, 2).reshape(128, 48)
    )
    return {
        "wq": wq_t, "bq": bq_a, "wk": wk_t, "bk": bk_a, "wv": wv_t, "bv": bv_a,
        "wfc": wfc_t, "bfc": bfc_a, "wpj": wpj_t, "bp": bp_a,
        "lng": lng_a, "lnb": lnb_a,
    }


_NC_CACHE = {}


def kernel(**inputs):
    if "nc" not in _NC_CACHE:
        _NC_CACHE["nc"] = _build_nc()
    nc = _NC_CACHE["nc"]

    x = np.asarray(inputs["x"], dtype=np.float32)
    query = np.asarray(inputs["query"], dtype=np.float32)
    shared = _prep_weights(
        np.asarray(inputs["wq"]), np.asarray(inputs["bq"]),
        np.asarray(inputs["wk"]), np.asarray(inputs["bk"]),
        np.asarray(inputs["wv"]), np.asarray(inputs["bv"]),
        np.asarray(inputs["w_fc"]), np.asarray(inputs["b_fc"]),
        np.asarray(inputs["w_proj"]), np.asarray(inputs["b_proj"]),
        np.asarray(inputs["ln_g"]), np.asarray(inputs["ln_b"]),
    )

    in_maps = []
    for b in range(B):
        zq = np.ascontiguousarray(
            np.concatenate([query[b, :, :, 0].T, query[b, :, :, 1].T], axis=0)
            .reshape(KC_D, 128, T)
        )
        zx = np.ascontiguousarray(
            np.concatenate([x[b, :, :, 0].T, x[b, :, :, 1].T], axis=0)
            .reshape(KC_D, 128, T)
        )
        m = {"zq": zq, "zx": zx}
        m.update(shared)
        in_maps.append(m)

    import os
    trace = bool(os.environ.get("KERNEL_TRACE"))
    res = run_bass_kernel_spmd(nc, in_maps, list(range(N_CORES)), trace=trace)
    _NC_CACHE["exec_time_ns"] = res.exec_time_ns
    out = np.empty((B, S, D, 2), dtype=np.float32)
    for b in range(B):
        yb = res.results[b]["y"].reshape(D2, T)
        out[b, :, :, 0] = yb[:D, :].T
        out[b, :, :, 1] = yb[D:, :].T
    return out


if __name__ == "__main__":
    rng = np.random.default_rng(0)
    f = np.float32
    demo = {
        "x": rng.standard_normal((B, S, D, 2), dtype=f),
        "query": rng.standard_normal((B, S, D, 2), dtype=f),
        "wq": rng.standard_normal((D, D, 2), dtype=f) * 0.02,
        "bq": rng.standard_normal((D, 2), dtype=f) * 0.02,
        "wk": rng.standard_normal((D, D, 2), dtype=f) * 0.02,
        "bk": rng.standard_normal((D, 2), dtype=f) * 0.02,
        "wv": rng.standard_normal((D, D, 2), dtype=f) * 0.02,
        "bv": rng.standard_normal((D, 2), dtype=f) * 0.02,
        "w_fc": rng.standard_normal((HID, D, 2), dtype=f) * 0.02,
        "b_fc": rng.standard_normal((HID, 2), dtype=f) * 0.02,
        "w_proj": rng.standard_normal((D, HID, 2), dtype=f) * 0.02,
        "b_proj": rng.standard_normal((D, HID * 0 + 2), dtype=f) * 0.02,
        "ln_g": np.ones((3, 2, D), dtype=f),
        "ln_b": np.zeros((3, 2, D), dtype=f),
    }
    out = kernel(**demo)
    print("out shape", out.shape)



# revision 12
# speedup vs baseline: 1.5625x; 1.5625x over previous
"""Trainium2 Bass kernel for nn_ComplexCrossAttention.

Strategy (v2):
- Data-parallel over batch B=8 across 8 NeuronCores (one batch element each,
  no collectives).
- All matmul operands are bf16 (PSUM accumulation fp32): enables FWL so
  LDWEIGHTS overlaps matmuls, and halves weight DMA vs fp32.
- QKV projections stay in the stacked-real form Z=[re;im] with prestacked
  weights; the complex MLP uses the Gauss 3-multiplication trick
  (T1=Ar Wr, T2=Ai Wi, T3=(Ar+Ai)(Wr+Wi)) cutting c_fc/c_proj PE time 25%.
- Attention per head: transposed scores St[k,q], exp straight out of PSUM,
  key-axis softmax sums via ones-matmuls, 1/denom via reciprocal_approx_fast,
  normalization + V-bias + query-residual folded into the AV eviction.
  The V bias is deferred through softmax (attention rows sum to 1):
  obias_r = bvr - bvi, obias_i = bvr + bvi added at eviction.
- Activations/weights shipped in partition-major contiguous layouts so all
  big DMAs are linear.
"""

import sys

for _p in ("/opt/trn_rl_repo",):
    if _p not in sys.path:
        sys.path.insert(0, _p)

import numpy as np
import ml_dtypes

import concourse.bass as bass
import concourse.mybir as mybir
import concourse.tile as tile
from concourse import bacc
from concourse.bass_utils import run_bass_kernel_spmd

BF16 = mybir.dt.bfloat16
FP32 = mybir.dt.float32
AF = mybir.ActivationFunctionType
OP = mybir.AluOpType

B, S, D = 8, 512, 1024
NH, DH = 16, 64
HID = 4096
T = S
N_CORES = 8
D2 = 2 * D       # 2048 stacked features
KC_D = D2 // 128   # 16 contraction chunks of the model dim
MC_D = D2 // 128   # 16 chunks of the model dim
OC_H = HID // 128  # 32 out chunks of one MLP hidden component
KC_H = HID // 128  # 32 contraction chunks of one hidden component
EPS = 1e-5
NPBF = ml_dtypes.bfloat16


def _build_nc():
    nc = bacc.Bacc(None, target_bir_lowering=False, debug=False)

    zq_d = nc.dram_tensor("zq", [128, KC_D, T], BF16, kind="ExternalInput")
    zx_d = nc.dram_tensor("zx", [128, KC_D, T], BF16, kind="ExternalInput")
    wq_d = nc.dram_tensor("wq", [MC_D, 128, KC_D, 128], BF16, kind="ExternalInput")
    wk_d = nc.dram_tensor("wk", [MC_D, 128, KC_D, 128], BF16, kind="ExternalInput")
    wv_d = nc.dram_tensor("wv", [NH // 2, 128, KC_D, 256], BF16, kind="ExternalInput")
    wfc_d = nc.dram_tensor("wfc", [3, OC_H, 128, 8, 128], BF16, kind="ExternalInput")
    wpj_d = nc.dram_tensor("wpj", [3, 8, 128, KC_H, 128], BF16, kind="ExternalInput")
    bq_d = nc.dram_tensor("bq", [128, MC_D], FP32, kind="ExternalInput")
    bk_d = nc.dram_tensor("bk", [128, MC_D], FP32, kind="ExternalInput")
    ob_d = nc.dram_tensor("ob", [128, NH], FP32, kind="ExternalInput")
    bfc_d = nc.dram_tensor("bfc", [128, 2 * OC_H], FP32, kind="ExternalInput")
    bp_d = nc.dram_tensor("bp", [128, 16], FP32, kind="ExternalInput")
    lng_d = nc.dram_tensor("lng", [128, 48], FP32, kind="ExternalInput")
    lnb_d = nc.dram_tensor("lnb", [128, 48], FP32, kind="ExternalInput")
    y_d = nc.dram_tensor("y", [128, MC_D, T], FP32, kind="ExternalOutput")

    with tile.TileContext(nc) as tc:
        consts_cm = tc.tile_pool(name="consts", bufs=1)
        consts = consts_cm.__enter__()

        ones_b = consts.tile([128, 1], BF16)
        nc.vector.memset(ones_b[:], 1.0)
        eps_t = consts.tile([128, 1], FP32)
        nc.vector.memset(eps_t[:], EPS)
        bq_s = consts.tile([128, MC_D], FP32)
        nc.sync.dma_start(bq_s[:], bq_d[:])
        bk_s = consts.tile([128, MC_D], FP32)
        nc.sync.dma_start(bk_s[:], bk_d[:])
        ob_s = consts.tile([128, NH], FP32)
        nc.sync.dma_start(ob_s[:], ob_d[:])
        bfc_s = consts.tile([128, 2 * OC_H], FP32)
        nc.sync.dma_start(bfc_s[:], bfc_d[:])
        bp_s = consts.tile([128, 16], FP32)
        nc.sync.dma_start(bp_s[:], bp_d[:])
        lng_s = consts.tile([128, 48], FP32)
        nc.sync.dma_start(lng_s[:], lng_d[:])
        lnb_s = consts.tile([128, 48], FP32)
        nc.sync.dma_start(lnb_s[:], lnb_d[:])

        def ln_gb(idx, comp, c8):
            j = idx * 16 + comp * 8 + c8
            return lng_s[:, j:j + 1], lnb_s[:, j:j + 1]

        # ---- long-lived activation pools (manually scoped, LIFO order:
        # entered in reverse order of release) ----
        yp_cm = tc.tile_pool(name="yp", bufs=1)
        yp_pool = yp_cm.__enter__()
        y_pre = yp_pool.tile([128, MC_D, T], BF16, name="y_pre")

        x2n_cm = tc.tile_pool(name="x2n", bufs=1)
        x2n_pool = x2n_cm.__enter__()
        x2n = x2n_pool.tile([128, MC_D, T], BF16, name="x2n")

        zx_cm = tc.tile_pool(name="zx", bufs=1)
        zx_pool = zx_cm.__enter__()
        zx_s = zx_pool.tile([128, KC_D, T], BF16, name="zx_s")
        for i in range(4):
            nc.sync.dma_start(
                zx_s[:, i * 4:(i + 1) * 4, :], zx_d[:, i * 4:(i + 1) * 4, :]
            )

        zq_cm = tc.tile_pool(name="zq", bufs=1)
        zq_pool = zq_cm.__enter__()
        zq_s = zq_pool.tile([128, KC_D, T], BF16, name="zq_s")
        for i in range(4):
            nc.sync.dma_start(
                zq_s[:, i * 4:(i + 1) * 4, :], zq_d[:, i * 4:(i + 1) * 4, :]
            )

        o_cm = tc.tile_pool(name="op", bufs=1)
        o_pool = o_cm.__enter__()
        o_s = o_pool.tile([128, MC_D, T], BF16, name="o_s")

        q_cm = tc.tile_pool(name="qp", bufs=1)
        q_pool = q_cm.__enter__()
        q_s = q_pool.tile([128, NH, T], BF16, name="q_s")

        # =============== Phase A: Q projection (feature-major) ===============
        with (
            tc.tile_pool(name="wqp", bufs=3) as wq_pool,
            tc.tile_pool(name="psA", bufs=4, space="PSUM") as psA,
        ):
            for mc in range(MC_D):
                wt = wq_pool.tile([128, KC_D, 128], BF16, tag="wq")
                nc.sync.dma_start(wt[:], wq_d[mc])
                ps = psA.tile([128, T], FP32, tag="psA")
                for kc in range(KC_D):
                    nc.tensor.matmul(
                        ps[:], wt[:, kc, :], zq_s[:, kc, :],
                        start=(kc == 0), stop=(kc == KC_D - 1),
                    )
                nc.scalar.activation(
                    q_s[:, mc, :], ps[:], AF.Identity, bias=bq_s[:, mc:mc + 1]
                )

        # =============== Phase B: attention, head-streamed ===============
        with (
            tc.tile_pool(name="wkp", bufs=2) as wk_pool,
            tc.tile_pool(name="wvp", bufs=2) as wv_pool,
            tc.tile_pool(name="kp", bufs=4) as k_pool,
            tc.tile_pool(name="vp", bufs=2) as v_pool,
            tc.tile_pool(name="ep", bufs=10) as e_pool,
            tc.tile_pool(name="ttp", bufs=4) as tt_pool,
            tc.tile_pool(name="stp", bufs=2) as st_pool,
            tc.tile_pool(name="recp", bufs=4) as rec_pool,
            tc.tile_pool(name="bcp", bufs=4) as bc_pool,
            tc.tile_pool(name="psK", bufs=1, space="PSUM") as psK,
            tc.tile_pool(name="psV", bufs=1, space="PSUM") as psV,
            tc.tile_pool(name="psS", bufs=2, space="PSUM") as psS,
            tc.tile_pool(name="psO", bufs=2, space="PSUM") as psO,
            tc.tile_pool(name="psD", bufs=2, space="PSUM") as psD,
        ):
            v_cur = None
            for h in range(NH):
                hp, par = divmod(h, 2)
                if par == 0:
                    # V projection for the head pair (token-major), no bias
                    # (deferred through softmax into obias at eviction).
                    wvt = wv_pool.tile([128, KC_D, 256], BF16, tag="wv")
                    nc.sync.dma_start(wvt[:], wv_d[hp])
                    v_cur = v_pool.tile([128, 4, 512], BF16, tag="v")
                    for tcb in range(4):
                        psv = psV.tile([128, 256], FP32, tag="psV")
                        for kc in range(KC_D):
                            nc.tensor.matmul(
                                psv[:],
                                zx_s[:, kc, tcb * 128:(tcb + 1) * 128],
                                wvt[:, kc, :],
                                start=(kc == 0), stop=(kc == KC_D - 1),
                            )
                        for sub in range(2):
                            base = sub * 256
                            # V1 = [Vr | Vi]
                            nc.vector.tensor_copy(
                                v_cur[:, tcb, base:base + 128],
                                psv[:, sub * 128:(sub + 1) * 128],
                            )
                            # V2 = [-Vi | Vr]
                            nc.scalar.activation(
                                v_cur[:, tcb, base + 128:base + 192],
                                psv[:, sub * 128 + 64:sub * 128 + 128],
                                AF.Identity, scale=-1.0,
                            )
                            nc.scalar.activation(
                                v_cur[:, tcb, base + 192:base + 256],
                                psv[:, sub * 128:sub * 128 + 64],
                                AF.Copy,
                            )

                # K1 = [Kr; -Ki] projection (feature-major); K2 = [Ki; Kr]
                wkt = wk_pool.tile([128, KC_D, 128], BF16, tag="wk")
                nc.sync.dma_start(wkt[:], wk_d[h])
                k1 = k_pool.tile([128, T], BF16, tag="k")
                ps = psK.tile([128, T], FP32, tag="psK")
                for kc in range(KC_D):
                    nc.tensor.matmul(
                        ps[:], wkt[:, kc, :], zx_s[:, kc, :],
                        start=(kc == 0), stop=(kc == KC_D - 1),
                    )
                nc.scalar.activation(
                    k1[:], ps[:], AF.Identity, bias=bk_s[:, h:h + 1]
                )
                k2 = k_pool.tile([128, T], BF16, tag="k")
                nc.sync.dma_start(k2[0:64, :], k1[64:128, :])
                nc.vector.tensor_scalar_mul(k2[0:64, :], k2[0:64, :], -1.0)
                nc.sync.dma_start(k2[64:128, :], k1[0:64, :])
                k_t = [k1, k2]

                # transposed scores + exp (comp 0: re via K1, comp 1: im via K2)
                e_tiles = [[None] * 4 for _ in range(2)]
                for comp in range(2):
                    for kc4 in range(4):
                        pss = psS.tile([128, T], FP32, tag="psS")
                        nc.tensor.matmul(
                            pss[:],
                            k_t[comp][:, kc4 * 128:(kc4 + 1) * 128],
                            q_s[:, h, :],
                            start=True, stop=True,
                        )
                        et = e_pool.tile([128, T], BF16, tag="e")
                        nc.scalar.activation(et[:], pss[:], AF.Exp)
                        e_tiles[comp][kc4] = et

                # softmax denominators -> fast reciprocal -> broadcast
                bc = []
                for comp in range(2):
                    psd = psD.tile([1, T], FP32, tag="psD")
                    for kc4 in range(4):
                        nc.tensor.matmul(
                            psd[:], ones_b[:], e_tiles[comp][kc4],
                            start=(kc4 == 0), stop=(kc4 == 3),
                        )
                    rec = rec_pool.tile([1, T], FP32, tag="rec")
                    nc.vector.reciprocal_approx_fast(rec[:], psd[:])
                    bct = bc_pool.tile([128, T], FP32, tag="bc")
                    nc.gpsimd.partition_broadcast(bct[:], rec[:])
                    bc.append(bct)

                # AV: two accumulation groups (er-part needs /dr, ei-part /di)
                pso = []
                for comp in range(2):
                    p = psO.tile([128, T], FP32, tag="psO")
                    for kc4 in range(4):
                        base = par * 256 + comp * 128
                        nc.tensor.matmul(
                            p[:],
                            v_cur[:, kc4, base:base + 128],
                            e_tiles[comp][kc4],
                            start=(kc4 == 0), stop=(kc4 == 3),
                        )
                    pso.append(p)

                # eviction: comb = pso0/d_r + pso1/d_i + obias; rows
                # [Or(0:64); Oi(64:128)]; query residual fused here.
                c = h // 2
                ta = tt_pool.tile([128, T], FP32, tag="ta")
                tb = tt_pool.tile([128, T], FP32, tag="tb")
                comb = tt_pool.tile([128, T], BF16, tag="comb")
                nc.vector.tensor_tensor(ta[:], pso[0][:], bc[0][:], OP.mult)
                nc.vector.tensor_tensor(tb[:], pso[1][:], bc[1][:], OP.mult)
                nc.vector.scalar_tensor_tensor(
                    comb[:], ta[:], ob_s[:, h:h + 1], tb[:], OP.add, OP.add
                )
                if par == 0:
                    dsl, cc = slice(0, 64), c          # direct Or
                    ssl, sc = slice(64, 128), 8 + c    # staged Oi
                    msl = slice(0, 64)
                else:
                    dsl, cc = slice(64, 128), 8 + c    # direct Oi
                    ssl, sc = slice(0, 64), c          # staged Or
                    msl = slice(64, 128)
                nc.vector.tensor_tensor(
                    o_s[dsl, cc, :], comb[dsl, :], zq_s[dsl, cc, :], OP.add
                )
                stg = st_pool.tile([128, T], BF16, tag="stg")
                nc.sync.dma_start(stg[msl, :], comb[ssl, :])
                nc.vector.tensor_tensor(
                    o_s[msl, sc, :], stg[msl, :], zq_s[msl, sc, :], OP.add
                )

        q_cm.__exit__(None, None, None)

        # =============== LayerNorm helper ===============
        def layer_norm(src_fn, dst_fn, idx, psum_pool, small, bcast, sqp,
                       res_fn=None, fp32_out=False, post_fn=None):
            """LN over the 1024 features of each of re (chunks 0-7) and im
            (chunks 8-15). If res_fn is given, dst = res + LN(src)."""
            ps_mean = []
            ps_sq = []
            for comp in range(2):
                pm = psum_pool.tile([1, T], FP32, tag="lnpm")
                for c8 in range(8):
                    nc.tensor.matmul(
                        pm[:], ones_b[:], src_fn(comp * 8 + c8),
                        start=(c8 == 0), stop=(c8 == 7),
                    )
                ps_mean.append(pm)
                pq = psum_pool.tile([1, T], FP32, tag="lnpq")
                for c8 in range(8):
                    sq = sqp.tile([128, T], BF16, tag="sq")
                    nc.scalar.activation(sq[:], src_fn(comp * 8 + c8), AF.Square)
                    nc.tensor.matmul(
                        pq[:], ones_b[:], sq[:],
                        start=(c8 == 0), stop=(c8 == 7),
                    )
                ps_sq.append(pq)
            bcs = []
            for comp in range(2):
                mean = small.tile([1, T], FP32, tag="mean")
                nc.vector.tensor_scalar_mul(mean[:], ps_mean[comp][:], 1.0 / D)
                msq = small.tile([1, T], FP32, tag="msq")
                nc.vector.tensor_scalar_mul(msq[:], ps_sq[comp][:], 1.0 / D)
                m2 = small.tile([1, T], FP32, tag="m2")
                nc.vector.tensor_tensor(m2[:], mean[:], mean[:], OP.mult)
                var = small.tile([1, T], FP32, tag="var")
                nc.vector.tensor_tensor(var[:], msq[:], m2[:], OP.subtract)
                sstd = small.tile([1, T], FP32, tag="sstd")
                nc.scalar.activation(sstd[:], var[:], AF.Sqrt, bias=eps_t[0:1, :])
                rstd = small.tile([1, T], FP32, tag="rstd")
                nc.vector.reciprocal_approx_fast(rstd[:], sstd[:])
                mr = small.tile([1, T], FP32, tag="mr")
                nc.vector.tensor_tensor(mr[:], mean[:], rstd[:], OP.mult)
                br = bcast.tile([128, T], FP32, tag="br")
                nc.gpsimd.partition_broadcast(br[:], rstd[:])
                bm = bcast.tile([128, T], FP32, tag="bm")
                nc.gpsimd.partition_broadcast(bm[:], mr[:])
                bcs.append((br, bm))
            for c in range(MC_D):
                comp = c // 8
                br, bm = bcs[comp]
                g_ap, b_ap = ln_gb(idx, comp, c % 8)
                t1 = sqp.tile([128, T], FP32, tag="lnt1")
                nc.vector.tensor_tensor(t1[:], src_fn(c), br[:], OP.mult)
                vhat = sqp.tile([128, T], FP32, tag="lnvh")
                nc.vector.tensor_tensor(vhat[:], t1[:], bm[:], OP.subtract)
                if res_fn is None:
                    nc.vector.tensor_scalar(
                        dst_fn(c), vhat[:], g_ap, b_ap, OP.mult, OP.add
                    )
                else:
                    t2 = sqp.tile([128, T], FP32 if fp32_out else BF16, tag="lnt2")
                    nc.vector.tensor_scalar(
                        t2[:], vhat[:], g_ap, b_ap, OP.mult, OP.add
                    )
                    nc.vector.tensor_tensor(dst_fn(c), t2[:], res_fn(c), OP.add)
                if post_fn is not None:
                    post_fn(c)

        # =============== Phase C: two layernorms ===============
        with (
            tc.tile_pool(name="lnsq", bufs=4) as sq_pool,
            tc.tile_pool(name="lnsm", bufs=1) as small_pool,
            tc.tile_pool(name="lnbc", bufs=4) as bc2_pool,
            tc.tile_pool(name="psC", bufs=4, space="PSUM") as psC,
        ):
            # LN#0 over (attn_out + query) [already fused], + x residual,
            # written into zx_s (x2pre)
            layer_norm(
                lambda c: o_s[:, c, :], lambda c: zx_s[:, c, :],
                0, psC, small_pool, bc2_pool, sq_pool,
                res_fn=lambda c: zx_s[:, c, :],
            )
            # LN#1 over x2pre -> x2n
            layer_norm(
                lambda c: zx_s[:, c, :], lambda c: x2n[:, c, :],
                1, psC, small_pool, bc2_pool, sq_pool,
            )

        o_cm.__exit__(None, None, None)
        zq_cm.__exit__(None, None, None)
        zx_cm.__exit__(None, None, None)

        # =============== Phase D: complex MLP (Gauss 3-mult) ===============
        with (
            tc.tile_pool(name="xsump", bufs=1) as xsum_pool,
            tc.tile_pool(name="hp", bufs=1) as h_pool,
            tc.tile_pool(name="wfcp", bufs=6) as wfc_pool,
            tc.tile_pool(name="wpjp", bufs=3) as wpj_pool,
            tc.tile_pool(name="mrt", bufs=2) as mr_pool,
            tc.tile_pool(name="psF", bufs=6, space="PSUM") as psF,
        ):
            xsum = xsum_pool.tile([128, 8, T], BF16, name="xsum")
            for c8 in range(8):
                nc.vector.tensor_tensor(
                    xsum[:, c8, :], x2n[:, c8, :], x2n[:, 8 + c8, :], OP.add
                )

            hr_t = h_pool.tile([128, OC_H, T], BF16, name="hr")
            hi_t = h_pool.tile([128, OC_H, T], BF16, name="hi")
            hs_t = h_pool.tile([128, OC_H, T], BF16, name="hs")

            # c_fc: per out chunk, three Gauss matmul groups
            for oc in range(OC_H):
                wts = []
                pss = []
                for g in range(3):
                    wt = wfc_pool.tile([128, 8, 128], BF16, tag="wfc")
                    nc.sync.dma_start(wt[:], wfc_d[g, oc])
                    wts.append(wt)
                    p = psF.tile([128, T], FP32, tag="psF")
                    src_base = (0, 8, 0)[g]
                    src = x2n if g < 2 else xsum
                    for kc in range(8):
                        nc.tensor.matmul(
                            p[:], wt[:, kc, :],
                            (src[:, src_base + kc, :] if g < 2
                             else xsum[:, kc, :]),
                            start=(kc == 0), stop=(kc == 7),
                        )
                    pss.append(p)
                # Hr = (T1 + br) - T2 ; Hi = ((T3 + bi) - T1) - T2
                # (DVE reads at most one PSUM operand: evict T1 via Act first)
                t1sb = mr_pool.tile([128, T], FP32, tag="t1sb")
                nc.scalar.activation(t1sb[:], pss[0][:], AF.Copy)
                nc.vector.scalar_tensor_tensor(
                    hr_t[:, oc, :], t1sb[:], bfc_s[:, oc:oc + 1], pss[1][:],
                    OP.add, OP.subtract,
                )
                tmp = mr_pool.tile([128, T], FP32, tag="gtmp")
                nc.vector.scalar_tensor_tensor(
                    tmp[:], pss[2][:], bfc_s[:, OC_H + oc:OC_H + oc + 1],
                    t1sb[:], OP.add, OP.subtract,
                )
                nc.vector.tensor_tensor(
                    hi_t[:, oc, :], tmp[:], pss[1][:], OP.subtract
                )
                # modReLU: hr += |h| (0.5 folded into wpj); hs = hr' + hi
                sq1 = mr_pool.tile([128, T], FP32, tag="mr1")
                nc.scalar.activation(sq1[:], hr_t[:, oc, :], AF.Square)
                sq2 = mr_pool.tile([128, T], FP32, tag="mr2")
                nc.scalar.activation(sq2[:], hi_t[:, oc, :], AF.Square)
                nc.vector.tensor_tensor(sq1[:], sq1[:], sq2[:], OP.add)
                mag = mr_pool.tile([128, T], BF16, tag="mag")
                nc.scalar.activation(mag[:], sq1[:], AF.Sqrt)
                nc.vector.tensor_tensor(
                    hr_t[:, oc, :], hr_t[:, oc, :], mag[:], OP.add
                )
                nc.vector.tensor_tensor(
                    hs_t[:, oc, :], hr_t[:, oc, :], hi_t[:, oc, :], OP.add
                )

            # c_proj: per out chunk pc, U1/U2/U3 Gauss groups; final
            # bias + x2n residual fused into eviction -> y_pre
            for pc in range(8):
                ups = []
                for g, hsrc in ((0, hr_t), (1, hi_t), (2, hs_t)):
                    wt = wpj_pool.tile([128, KC_H, 128], BF16, tag="wpj")
                    nc.sync.dma_start(wt[:], wpj_d[g, pc])
                    p = psF.tile([128, T], FP32, tag="psF")
                    for kc in range(KC_H):
                        nc.tensor.matmul(
                            p[:], wt[:, kc, :], hsrc[:, kc, :],
                            start=(kc == 0), stop=(kc == KC_H - 1),
                        )
                    ups.append(p)
                # Mr = (U1 + bpr) - U2 (+ x2n_r)
                u1sb = mr_pool.tile([128, T], FP32, tag="u1sb")
                nc.scalar.activation(u1sb[:], ups[0][:], AF.Copy)
                tmp = mr_pool.tile([128, T], BF16, tag="gtmp2")
                nc.vector.scalar_tensor_tensor(
                    tmp[:], u1sb[:], bp_s[:, pc:pc + 1], ups[1][:],
                    OP.add, OP.subtract,
                )
                nc.vector.tensor_tensor(
                    y_pre[:, pc, :], tmp[:], x2n[:, pc, :], OP.add
                )
                # Mi = ((U3 + bpi) - U1) - U2 (+ x2n_i)
                tmp2 = mr_pool.tile([128, T], FP32, tag="gtmp3")
                nc.vector.scalar_tensor_tensor(
                    tmp2[:], ups[2][:], bp_s[:, 8 + pc:8 + pc + 1], u1sb[:],
                    OP.add, OP.subtract,
                )
                tmp3 = mr_pool.tile([128, T], BF16, tag="gtmp4")
                nc.vector.tensor_tensor(tmp3[:], tmp2[:], ups[1][:], OP.subtract)
                nc.vector.tensor_tensor(
                    y_pre[:, 8 + pc, :], tmp3[:], x2n[:, 8 + pc, :], OP.add
                )

        x2n_cm.__exit__(None, None, None)

        # =============== final layernorm + store ===============
        with (
            tc.tile_pool(name="lnsq2", bufs=4) as sq2_pool,
            tc.tile_pool(name="lnsm2", bufs=1) as small2_pool,
            tc.tile_pool(name="lnbc2", bufs=4) as bc3_pool,
            tc.tile_pool(name="yt", bufs=4) as yt_pool,
            tc.tile_pool(name="psC2", bufs=4, space="PSUM") as psC2,
        ):
            yts = {}

            def y_dst(c):
                yt = yt_pool.tile([128, T], FP32, tag="yt")
                yts[c] = yt
                return yt[:]

            layer_norm(
                lambda c: y_pre[:, c, :], y_dst,
                2, psC2, small2_pool, bc3_pool, sq2_pool,
                post_fn=lambda c: nc.sync.dma_start(y_d[:, c, :], yts[c][:]),
            )

        yp_cm.__exit__(None, None, None)
        consts_cm.__exit__(None, None, None)

    nc.compile()
    if not nc.is_finalized():
        nc.finalize()
    return nc


def _stackT(w):
    """[F, Din, 2] torch-layout complex weight -> [2*Din, 2*F] stacked lhsT."""
    wr = w[..., 0].astype(np.float32)
    wi = w[..., 1].astype(np.float32)
    top = np.concatenate([wr.T, wi.T], axis=1)
    bot = np.concatenate([-wi.T, wr.T], axis=1)
    return np.concatenate([top, bot], axis=0)


def _prep_weights(wq, bq, wk, bk, wv, bv, w_fc, b_fc, w_proj, b_proj, ln_g, ln_b):
    qcols = np.concatenate(
        [np.concatenate([np.arange(h * 64, h * 64 + 64),
                         1024 + np.arange(h * 64, h * 64 + 64)]) for h in range(NH)]
    )
    scale = np.float32(1.0 / np.sqrt(DH))

    sq = _stackT(wq) * scale
    wq_t = np.ascontiguousarray(
        sq[:, qcols].reshape(KC_D, 128, MC_D, 128).transpose(2, 1, 0, 3)
    ).astype(NPBF)
    bq_l = (np.concatenate([bq[:, 0], bq[:, 1]]) * scale)[qcols]
    bq_a = np.ascontiguousarray(
        bq_l.reshape(MC_D, 128).T.astype(np.float32)
    )

    sk = _stackT(wk)
    bkst = np.concatenate([bk[:, 0], bk[:, 1]]).astype(np.float32)
    wk_full = sk[:, qcols].copy()           # [2048, 2048]: per head [Kr | Ki]
    bk_l = bkst[qcols].copy()
    for h in range(NH):
        wk_full[:, h * 128 + 64:h * 128 + 128] *= -1.0   # -> [Kr | -Ki]
        bk_l[h * 128 + 64:h * 128 + 128] *= -1.0
    wk_t = np.ascontiguousarray(
        wk_full.reshape(KC_D, 128, MC_D, 128).transpose(2, 1, 0, 3)
    ).astype(NPBF)
    bk_a = np.ascontiguousarray(bk_l.reshape(MC_D, 128).T.astype(np.float32))

    sv = _stackT(wv)
    svq = sv[:, qcols]                       # [2048, 2048]
    wv_t = np.ascontiguousarray(
        svq.reshape(KC_D, 128, NH // 2, 256).transpose(2, 1, 0, 3)
    ).astype(NPBF)
    # obias: V bias deferred through softmax; per head column:
    # rows 0:64 = bvr - bvi (Or), rows 64:128 = bvr + bvi (Oi)
    ob = np.empty((128, NH), dtype=np.float32)
    bvr, bvi = bv[:, 0].astype(np.float32), bv[:, 1].astype(np.float32)
    for h in range(NH):
        sl = slice(h * 64, h * 64 + 64)
        ob[0:64, h] = bvr[sl] - bvi[sl]
        ob[64:128, h] = bvr[sl] + bvi[sl]

    # Gauss c_fc: blocks Wr^T, Wi^T, (Wr+Wi)^T  [1024, 4096]
    fr = w_fc[..., 0].astype(np.float32).T
    fi = w_fc[..., 1].astype(np.float32).T
    wfc_t = np.ascontiguousarray(
        np.stack([fr, fi, fr + fi])
        .reshape(3, 8, 128, OC_H, 128).transpose(0, 3, 2, 1, 4)
    ).astype(NPBF)
    bfc_a = np.ascontiguousarray(
        np.concatenate([b_fc[:, 0], b_fc[:, 1]])
        .reshape(2 * OC_H, 128).T.astype(np.float32)
    )

    # Gauss c_proj (0.5 of modReLU folded into weights): [4096, 1024] blocks
    pr = (w_proj[..., 0].astype(np.float32) * 0.5).T
    pi = (w_proj[..., 1].astype(np.float32) * 0.5).T
    wpj_t = np.ascontiguousarray(
        np.stack([pr, pi, pr + pi])
        .reshape(3, KC_H, 128, 8, 128).transpose(0, 3, 2, 1, 4)
    ).astype(NPBF)
    bp_a = np.ascontiguousarray(
        np.concatenate([b_proj[:, 0], b_proj[:, 1]])
        .reshape(16, 128).T.astype(np.float32)
    )

    lng_a = np.ascontiguousarray(
        ln_g.astype(np.float32).reshape(3, 2, 8, 128).transpose(3, 0, 1, 2).reshape(128, 48)
    )
    lnb_a = np.ascontiguousarray(
        ln_b.astype(np.float32).reshape(3, 2, 8, 128).transpose(3, 0, 1, 2).reshape(128, 48)
    )
    return {
        "wq": wq_t, "bq": bq_a, "wk": wk_t, "bk": bk_a, "wv": wv_t, "ob": ob,
        "wfc": wfc_t, "bfc": bfc_a, "wpj": wpj_t, "bp": bp_a,
        "lng": lng_a, "lnb": lnb_a,
    }


_NC_CACHE = {}


def kernel(**inputs):
    if "nc" not in _NC_CACHE:
        _NC_CACHE["nc"] = _build_nc()
    nc = _NC_CACHE["nc"]

    x = np.asarray(inputs["x"], dtype=np.float32)
    query = np.asarray(inputs["query"], dtype=np.float32)
    shared = _prep_weights(
        np.asarray(inputs["wq"]), np.asarray(inputs["bq"]),
        np.asarray(inputs["wk"]), np.asarray(inputs["bk"]),
        np.asarray(inputs["wv"]), np.asarray(inputs["bv"]),
        np.asarray(inputs["w_fc"]), np.asarray(inputs["b_fc"]),
        np.asarray(inputs["w_proj"]), np.asarray(inputs["b_proj"]),
        np.asarray(inputs["ln_g"]), np.asarray(inputs["ln_b"]),
    )

    in_maps = []
    for b in range(B):
        zq = np.ascontiguousarray(
            np.concatenate([query[b, :, :, 0].T, query[b, :, :, 1].T], axis=0)
            .reshape(KC_D, 128, T).transpose(1, 0, 2)
        ).astype(NPBF)
        zx = np.ascontiguousarray(
            np.concatenate([x[b, :, :, 0].T, x[b, :, :, 1].T], axis=0)
            .reshape(KC_D, 128, T).transpose(1, 0, 2)
        ).astype(NPBF)
        m = {"zq": zq, "zx": zx}
        m.update(shared)
        in_maps.append(m)

    import os
    trace = bool(os.environ.get("KERNEL_TRACE"))
    res = run_bass_kernel_spmd(nc, in_maps, list(range(N_CORES)), trace=trace)
    _NC_CACHE["exec_time_ns"] = res.exec_time_ns
    out = np.empty((B, S, D, 2), dtype=np.float32)
    for b in range(B):
        yb = res.results[b]["y"].transpose(1, 0, 2).reshape(D2, T)
        out[b, :, :, 0] = yb[:D, :].T
        out[b, :, :, 1] = yb[D:, :].T
    return out


if __name__ == "__main__":
    rng = np.random.default_rng(0)
    f = np.float32
    demo = {
        "x": rng.standard_normal((B, S, D, 2), dtype=f),
        "query": rng.standard_normal((B, S, D, 2), dtype=f),
        "wq": rng.standard_normal((D, D, 2), dtype=f) * 0.02,
        "bq": rng.standard_normal((D, 2), dtype=f) * 0.02,
        "wk": rng.standard_normal((D, D, 2), dtype=f) * 0.02,
        "bk": rng.standard_normal((D, 2), dtype=f) * 0.02,
        "wv": rng.standard_normal((D, D, 2), dtype=f) * 0.02,
        "bv": rng.standard_normal((D, 2), dtype=f) * 0.02,
        "w_fc": rng.standard_normal((HID, D, 2), dtype=f) * 0.02,
        "b_fc": rng.standard_normal((HID, 2), dtype=f) * 0.02,
        "w_proj": rng.standard_normal((D, HID, 2), dtype=f) * 0.02,
        "b_proj": rng.standard_normal((D, 2), dtype=f) * 0.02,
        "ln_g": np.ones((3, 2, D), dtype=f),
        "ln_b": np.zeros((3, 2, D), dtype=f),
    }
    out = kernel(**demo)
    print("out shape", out.shape)
